# revision 34
# baseline (speedup 1.0000x reference)
# Trainium2 Bass kernel for nn_BERT_77008763617386 (dense_transformer).
#
# Sharding: pure data-parallel over batch. B=8 batch items -> 8 NeuronCores,
# one item per core. All weights replicated; no collectives. Host casts the
# large matmul weights to fp8(e4m3, x64 scaled) or fp16 and pre-arranges them
# in the SBUF layouts the kernel consumes.
#
# Device pipeline per core (S=512 tokens; activations kept feature-major
# [feat-part, token-free] between matmuls):
#   gather embeddings (indirect DMA, fp16 tables) -> LayerNorms via
#   bn_stats/bn_aggr -> PE transposes (identity matmul) into catT fp8
#   -> Wf via fp8 DoubleRow matmuls (2x PE rate)
#   -> qT/kT (fp8 DR) / v (fp8) -> transposed attention: scoresT computed
#      directly (kT stationary), exp -> a_fT fp16, row sums via ones-matmul
#      on PE, reciprocal once on [4,S], broadcast via K=4 matmul, folded
#      into ctxT copy
#   -> mha (+LN) -> FFN (ACT Gelu, W1 stationary so hdn emerges transposed)
#   -> (+LN) -> encT fp8 -> vocab head: fp8 DoubleRow matmuls vs x64 Wtok
#      (60 N=512 tiles, streamed twice over two s-halves), fused exp+row-sum
#      on ACT (exp discarded to fp8 scratch), raw x64 logits staged fp16 in
#      slab (DVE copy, some on ACT), log-softmax finalize = one fused
#      tensor_scalar (slab - 64*lsum)*(1/64) -> fp16 output DMA.
#
# DMA rings: scalar(HWDGE)=encoder weights/consts, sync(HWDGE)=wtok stream
# (ring of 20 tiles prefetches under the encoder), gpsimd(SWDGE)=embedding
# gathers + output stores (tail drain alternates gpsimd/sync).
#
# Numerics:
#  - fp8 e4m3 operands for Wf/q/k/v/vocab matmuls; weights scaled x64 into
#    e4m3 normal range, descaled in the PSUM->SBUF copies (or folded into
#    exp scale / the log-softmax finalize)
#  - attention softmax unnormalized through ctx; 1/rowsum folded into the
#    ctxT copy (LN downstream is per-token so the scale cannot be dropped)
#  - te's sqrt(1/DI) folded into its LN eps (LN is scale-invariant)
#  - vocab log-softmax without max-subtraction (logits bounded ~|3|)
#  - all bias vectors are structurally zero; attention_mask structurally
#    all-False (spec fill=zeros); both ignored
import math
from contextlib import ExitStack

import numpy as np
import ml_dtypes

B, S, V, PPOI, H, DI, DO = 8, 512, 30522, 10000, 4, 512, 128
P = 128
NT = S // P          # 4 token chunks of 128
KC = DI // P         # 4 k-tiles of the 512 feature dim
CATK = 5 * DI // P   # 20 k-tiles of the concat dim (16 device + 4 host pos)
NGF = 10             # Wf DoubleRow groups of 256 over the 2560 concat dim
NVT = 60             # vocab tiles of 512 (last ragged: 314)
VPAD = NVT * 512     # 30720
SQS = 1.0 / math.sqrt(float(S))
EPS = 1e-5
WSC = 64.0           # fp8 weight pre-scale
IWSC = 1.0 / WSC

# vocab output: 15 pieces of 2048 cols; all staged as descaled logits in
# the slab (DVE copy), finalize subs split between DVE and GpSimd
NPIECE = 15
PW = 2048

F8 = ml_dtypes.float8_e4m3

_CACHE: dict = {}


def _ln_np(x, eps=1e-5):
    m = x.mean(-1, keepdims=True)
    v = x.var(-1, keepdims=True)
    return (x - m) / np.sqrt(v + eps)


def _pair8(w, scale=WSC):
    """[K, N] fp32 -> [K//256, 128, 2, N] fp8 with row (2g+ko)*128+ki."""
    k, n = w.shape
    a = (scale * w).reshape(k // 256, 2, 128, n).transpose(0, 2, 1, 3)
    return np.ascontiguousarray(a).astype(F8)


def host_prep(inputs):
    """Cast/lay out weights and constants shared by all cores."""
    out = {}
    # LN'd positional encoding, feature-major k-tiles, fp8: [128, KC, S]
    dd = np.arange(DI)
    ang = np.arange(S, dtype=np.float32)[:, None] / (
        10000.0 ** (2.0 * dd / DI)
    )[None, :].astype(np.float32)
    pe = np.where(dd % 2 == 0, np.sin(ang), np.cos(ang)).astype(np.float32)
    pe_n = _ln_np(pe)  # [S, DI]
    out["pent8"] = np.ascontiguousarray(
        pe_n.T.reshape(KC, P, S).transpose(1, 0, 2)
    ).astype(F8)  # [128, KC, S]
    out["wtimeb"] = np.ascontiguousarray(
        np.broadcast_to(np.asarray(inputs["w_time"], np.float32), (P, DI))
    )
    # Wf rows permuted to device concat order [pure, te, semb, poi, pos]
    Wf = np.asarray(inputs["Wf"], np.float32)   # [H, 5DI, DI]
    perm = np.concatenate([
        np.arange(0, 512), np.arange(1024, 1536), np.arange(1536, 2048),
        np.arange(2048, 2560), np.arange(512, 1024)])
    WfP = Wf[:, perm, :]
    out["wf8"] = np.stack([_pair8(WfP[h]) for h in range(H)])  # [H,10,128,2,512]
    for nm, w in (("wq8", "Wq"), ("wk8", "Wk"), ("wv8", "Wv")):
        a = np.asarray(inputs[w], np.float32)
        out[nm] = np.ascontiguousarray(np.stack(
            [_pair8(a[h]) for h in range(H)]
        ).transpose(2, 0, 1, 3, 4))  # [128, H, 2, 2, DO]
    Wo = np.asarray(inputs["Wo"], np.float32).reshape(H, P, DI)
    out["wo16"] = np.ascontiguousarray(Wo.transpose(1, 0, 2)).astype(np.float16)
    W1 = np.asarray(inputs["W1"], np.float32).reshape(KC, P, DO)
    out["w116"] = np.ascontiguousarray(W1.transpose(1, 0, 2)).astype(np.float16)
    out["w216"] = np.asarray(inputs["W2"], np.float32).astype(np.float16)
    Wtok = np.asarray(inputs["Wtok"], np.float32)
    wtok_pad = np.zeros((DI, VPAD), np.float32)
    wtok_pad[:, :V] = Wtok
    # [NVT, 128, 2, 2, 512]: per vp, partition-major, the two DR k-groups
    a = (WSC * wtok_pad).reshape(2, 2, P, NVT, 512).transpose(3, 2, 0, 1, 4)
    out["wtok8"] = np.ascontiguousarray(a).astype(F8)
    out["semb_tab"] = np.asarray(inputs["s_emb_table"], np.float16)
    out["spat_tab"] = np.asarray(inputs["spatial_table"], np.float16)
    out["poi_tab"] = np.asarray(inputs["poi_table"], np.float16)
    # head-selector constants for attention row-sums / broadcast
    hsel = np.zeros((P, H * H), np.float16)
    for h in range(H):
        hsel[:, h * H + h] = 1.0
    out["hsel"] = hsel
    hbsel = np.zeros((H, H * P), np.float16)
    for h in range(H):
        hbsel[h, h * P:(h + 1) * P] = 1.0
    out["hbsel"] = hbsel
    out["ident16"] = np.eye(P, dtype=np.float16)
    return out


def host_prep_core(inputs, b):
    """Per-core (per batch item) inputs, wrapped [128, NT] partition-major."""
    wrap_i = lambda a: np.ascontiguousarray(
        np.asarray(a, np.int32).reshape(NT, P).T)
    return {
        "ids_w": wrap_i(inputs["input_tensor"][b]),
        "poi_w": wrap_i(inputs["poi_tensor"][b]),
        "time_w": np.ascontiguousarray(
            np.asarray(inputs["time_tensor"][b], np.float32).reshape(NT, P).T),
    }


def build_program():
    import concourse.bass as bass
    import concourse.mybir as mybir
    import concourse.tile as tile
    from concourse import bacc

    dt = mybir.dt
    AF = mybir.ActivationFunctionType
    OP = mybir.AluOpType
    AX = mybir.AxisListType
    DR = mybir.MatmulPerfMode.DoubleRow
    ts, ds = bass.ts, bass.ds

    nc = bacc.Bacc("TRN2", target_bir_lowering=False, debug=False,
                   enable_asserts=False)

    # ---- DRAM I/O ----
    ids_d = nc.dram_tensor("ids_w", [P, NT], dt.int32, kind="ExternalInput")
    poi_d = nc.dram_tensor("poi_w", [P, NT], dt.int32, kind="ExternalInput")
    time_d = nc.dram_tensor("time_w", [P, NT], dt.float32, kind="ExternalInput")
    semb_t = nc.dram_tensor("semb_tab", [V, DI], dt.float16, kind="ExternalInput")
    spat_t = nc.dram_tensor("spat_tab", [V, DI], dt.float16, kind="ExternalInput")
    poi_t = nc.dram_tensor("poi_tab", [PPOI, DI], dt.float16, kind="ExternalInput")
    pent_d = nc.dram_tensor("pent8", [P, KC, S], dt.float8e4, kind="ExternalInput")
    wtimeb_d = nc.dram_tensor("wtimeb", [P, DI], dt.float32, kind="ExternalInput")
    wf_d = nc.dram_tensor("wf8", [H, NGF, P, 2, DI], dt.float8e4,
                          kind="ExternalInput")
    wq_d = nc.dram_tensor("wq8", [P, H, 2, 2, DO], dt.float8e4, kind="ExternalInput")
    wk_d = nc.dram_tensor("wk8", [P, H, 2, 2, DO], dt.float8e4, kind="ExternalInput")
    wv_d = nc.dram_tensor("wv8", [P, H, 2, 2, DO], dt.float8e4, kind="ExternalInput")
    wo_d = nc.dram_tensor("wo16", [P, H, DI], dt.float16, kind="ExternalInput")
    w1_d = nc.dram_tensor("w116", [P, KC, DO], dt.float16, kind="ExternalInput")
    w2_d = nc.dram_tensor("w216", [P, DI], dt.float16, kind="ExternalInput")
    wtok_d = nc.dram_tensor("wtok8", [NVT, P, 2, 2, 512], dt.float8e4,
                            kind="ExternalInput")
    hsel_d = nc.dram_tensor("hsel", [P, H * H], dt.float16, kind="ExternalInput")
    hbsel_d = nc.dram_tensor("hbsel", [H, H * P], dt.float16,
                             kind="ExternalInput")
    ident_d = nc.dram_tensor("ident16", [P, P], dt.float16, kind="ExternalInput")
    out_d = nc.dram_tensor("out", [S, V], dt.float16, kind="ExternalOutput")

    with tile.TileContext(nc) as tc, ExitStack() as top:
        const = top.enter_context(tc.tile_pool(name="const", bufs=1))
        # idx/time first on the sync ring (ahead of the wtok prefetch flood)
        idx_sb = const.tile([P, 2 * NT], dt.int32)
        nc.sync.dma_start(idx_sb[:, 0:NT], ids_d[:])
        nc.sync.dma_start(idx_sb[:, NT:2 * NT], poi_d[:])
        time_sb = const.tile([P, NT], dt.float32)
        nc.sync.dma_start(time_sb[:], time_d[:])
        wtimeb_sb = const.tile([P, DI], dt.float32)
        nc.sync.dma_start(wtimeb_sb[:], wtimeb_d[:])
        halfpi = const.tile([P, 1], dt.float32)
        nc.gpsimd.memset(halfpi[:], math.pi / 2.0)
        eps1 = const.tile([P, 1], dt.float32)
        nc.gpsimd.memset(eps1[:], EPS)
        epste = const.tile([P, 1], dt.float32)
        nc.gpsimd.memset(epste[:], EPS * DI)
        iwsc_sb = const.tile([P, 1], dt.float32)
        nc.gpsimd.memset(iwsc_sb[:], IWSC)
        # const tiles allocated here; their scalar-ring DMAs issue inside
        # the encoder scope, after pent (program order = queue order)
        hsel_sb = const.tile([P, H * H], dt.float16)
        hbsel_sb = const.tile([H, H * P], dt.float16)
        ident_sb = const.tile([P, P], dt.float16)
        wq_sb = const.tile([P, H, 2, 2, DO], dt.float8e4)
        wk_sb = const.tile([P, H, 2, 2, DO], dt.float8e4)
        wv_sb = const.tile([P, H, 2, 2, DO], dt.float8e4)
        wo_sb = const.tile([P, H, DI], dt.float16)
        w1_sb = const.tile([P, KC, DO], dt.float16)
        w2_sb = const.tile([P, DI], dt.float16)

        encT_pool = top.enter_context(tc.tile_pool(name="encTp", bufs=1))
        encT8 = encT_pool.tile([P, KC, S], dt.float8e4)
        # wtok streaming ring lives at top level so prefetch can start early
        wtokp = top.enter_context(tc.tile_pool(name="wtokp", bufs=8))
        sumsp = top.enter_context(tc.tile_pool(name="sumsp", bufs=1))
        sums_sb = sumsp.tile([P, NT, NPIECE], dt.float32)

        # ======================= encoder =======================
        with ExitStack() as ectx:
            acts = ectx.enter_context(tc.tile_pool(name="acts", bufs=1))
            embp = ectx.enter_context(tc.tile_pool(name="embp", bufs=1))
            scrp = ectx.enter_context(tc.tile_pool(name="scrp", bufs=2))
            stat = ectx.enter_context(tc.tile_pool(name="stat", bufs=3))
            wfp = ectx.enter_context(tc.tile_pool(name="wfp", bufs=4))
            rsbp = ectx.enter_context(tc.tile_pool(name="rsbp", bufs=2))
            psA = ectx.enter_context(
                tc.tile_pool(name="psA", bufs=5, space="PSUM"))
            psT = ectx.enter_context(
                tc.tile_pool(name="psT", bufs=1, space="PSUM"))
            psS = ectx.enter_context(
                tc.tile_pool(name="psS", bufs=1, space="PSUM"))
            psB = ectx.enter_context(
                tc.tile_pool(name="psB", bufs=1, space="PSUM"))

            lnbuf = acts.tile([P, NT, 4, DI], dt.float16)
            catT8 = acts.tile([P, CATK, S], dt.float8e4)
            fusedT8 = acts.tile([P, H, KC, S], dt.float8e4)
            qT = acts.tile([P, H, S], dt.float16)
            kT = acts.tile([P, H, S], dt.float16)
            v_sb = acts.tile([P, NT, H, DO], dt.float16)
            afT = acts.tile([P, NT, H, S], dt.float16)
            ctxT = acts.tile([P, H, S], dt.float16)
            mha_n = acts.tile([P, NT, DI], dt.float16)
            mhaT16 = acts.tile([P, KC, S], dt.float16)
            hdnT = acts.tile([P, S], dt.float16)
            enc_n = acts.tile([P, NT, DI], dt.float16)

            # embedding gathers first (DMA latency); fp16 tables
            gath = {}
            for tab, idx_off, comp in (
                (spat_t, 0, 0), (semb_t, 0, 2), (poi_t, NT, 3),
            ):
                for c in range(NT):
                    g = embp.tile([P, DI], dt.float16, tag="g16", bufs=12)
                    nc.gpsimd.indirect_dma_start(
                        out=g[:], out_offset=None, in_=tab[:],
                        in_offset=bass.IndirectOffsetOnAxis(
                            ap=idx_sb[:, idx_off + c: idx_off + c + 1],
                            axis=0))
                    gath[(comp, c)] = g[:]

            # positional component: direct fp8 DMA into catT8 k-tiles 16..19
            # (first on the scalar ring — Wf's earliest groups need it)
            nc.scalar.dma_start(catT8[:, 16:CATK, :], pent_d[:])
            nc.scalar.dma_start(ident_sb[:], ident_d[:])
            nc.scalar.dma_start(wv_sb[:], wv_d[:])

            def ln_rows(xs, outs, eps_ap):
                """Row-LN NT tiles [128, DI] (SBUF or PSUM) -> outs via
                bn_stats/bn_aggr; eps_ap is a [P,1] bias tile. Normalizes
                split between DVE (tensor_scalar) and ACT (Copy w/ affine)."""
                bns = stat.tile([P, NT, 6], dt.float32, tag="bns")
                mvc = stat.tile([P, NT, 2], dt.float32, tag="mvc")
                for c in range(NT):
                    nc.vector.bn_stats(bns[:, c, :], xs[c])
                    nc.vector.bn_aggr(mvc[:, c, :], bns[:, c, :])
                std = stat.tile([P, NT, 1], dt.float32, tag="std")
                nc.scalar.activation(std[:], mvc[:, :, 1:2], AF.Sqrt,
                                     bias=eps_ap[:])
                inv = stat.tile([P, NT, 1], dt.float32, tag="inv")
                nc.vector.reciprocal(inv[:], std[:])
                negminv = stat.tile([P, NT, 1], dt.float32, tag="negminv")
                nc.vector.scalar_tensor_tensor(
                    out=negminv[:], in0=mvc[:, :, 0:1], scalar=-1.0,
                    in1=inv[:], op0=OP.mult, op1=OP.mult)
                for c in range(NT):
                    if c % 2 == 0:
                        nc.vector.tensor_scalar(
                            out=outs[c], in0=xs[c],
                            scalar1=mvc[:, c, 0:1], scalar2=inv[:, c, :],
                            op0=OP.subtract, op1=OP.mult)
                    else:
                        nc.scalar.activation(
                            outs[c], xs[c], AF.Identity,
                            scale=inv[:, c, :], bias=negminv[:, c, :])

            def finish_comp(comp):
                """PE-transpose lnbuf component -> catT8 fp8 k-tiles."""
                for kt in range(KC):
                    pst = psT.tile([P, S], dt.float16, tag="psT")
                    for c in range(NT):
                        nc.tensor.transpose(
                            pst[:, ts(c, P)],
                            lnbuf[:, c, comp, ts(kt, P)], ident_sb[:])
                    if kt % 2 == 0:
                        nc.vector.tensor_copy(catT8[:, comp * KC + kt, :],
                                              pst[:])
                    else:
                        nc.scalar.copy(catT8[:, comp * KC + kt, :], pst[:])

            sc_emb = nc.enter_named_scope("emb", False)
            # temporal component first (no gather dependency)
            te_xs = []
            for c in range(NT):
                angt = embp.tile([P, DI], dt.float32, tag="angf", bufs=2)
                nc.vector.tensor_scalar_mul(angt[:], wtimeb_sb[:],
                                            time_sb[:, c:c + 1])
                te = embp.tile([P, DI], dt.float16, tag="te16", bufs=4)
                nc.scalar.activation(te[:], angt[:], AF.Sin, bias=halfpi[:])
                te_xs.append(te)
            ln_rows([x[:] for x in te_xs],
                    [lnbuf[:, c, 1, :] for c in range(NT)], epste)
            finish_comp(1)
            # non-critical consts now (scalar ring, after the te chain so
            # they don't delay Sin/Sqrt at the head of the scalar queue)
            nc.scalar.dma_start(wq_sb[:], wq_d[:])
            nc.scalar.dma_start(wk_sb[:], wk_d[:])
            nc.scalar.dma_start(hsel_sb[:], hsel_d[:])
            nc.scalar.dma_start(hbsel_sb[:], hbsel_d[:])
            nc.scalar.dma_start(wo_sb[:], wo_d[:])
            nc.scalar.dma_start(w1_sb[:], w1_d[:])
            nc.scalar.dma_start(w2_sb[:], w2_d[:])
            # gathered components: pure(spatial)=0, semb=2, poi=3
            for comp in (0, 2, 3):
                xs = [gath[(comp, c)] for c in range(NT)]
                ln_rows([x[:] for x in xs],
                        [lnbuf[:, c, comp, :] for c in range(NT)], eps1)
                finish_comp(comp)
            nc.leave_named_scope("emb", sc_emb[0], False)

            sc_hd = nc.enter_named_scope("heads", False)
            # ---- per-head fusedT via fp8 DoubleRow ----
            # g-order by operand readiness: pos (DMA), te, pure, semb, poi
            GORDER = [8, 9, 2, 3, 0, 1, 4, 5, 6, 7]
            for h in range(H):
                psf = [psA.tile([P, S], dt.float32, tag="psA",
                                name=f"psf{h}_{i}") for i in range(KC)]
                for gi, g in enumerate(GORDER):
                    wf_t = wfp.tile([P, 2, DI], dt.float8e4, tag="wf")
                    # sync ring: wf slot-reuse waits must not head-of-line
                    # block the scalar/ACT queue (Sqrt etc. sit behind them)
                    nc.sync.dma_start(wf_t[:], wf_d[h, g])
                    for dtile in range(KC):
                        nc.tensor.matmul(
                            psf[dtile][:], wf_t[:, :, ds(dtile * P, P)],
                            catT8[:, ds(2 * g, 2), :], perf_mode=DR,
                            start=(gi == 0), stop=(gi == NGF - 1))
                for dtile in range(KC):
                    if dtile % 2 == 0:
                        nc.vector.tensor_scalar_mul(
                            fusedT8[:, h, dtile, :], psf[dtile][:], IWSC)
                    else:
                        nc.scalar.activation(
                            fusedT8[:, h, dtile, :], psf[dtile][:],
                            AF.Copy, scale=IWSC)
                for qi, (dst, w8) in enumerate(((qT, wq_sb), (kT, wk_sb))):
                    psq = psA.tile([P, S], dt.float32, tag="psA")
                    for g in range(2):
                        nc.tensor.matmul(
                            psq[:], w8[:, h, g], fusedT8[:, h, ds(2 * g, 2), :],
                            perf_mode=DR, start=(g == 0), stop=(g == 1))
                    if qi == 0:
                        nc.vector.tensor_scalar_mul(dst[:, h, :], psq[:], IWSC)
                    else:
                        nc.scalar.activation(dst[:, h, :], psq[:],
                                             AF.Copy, scale=IWSC)
            # ---- v (fp8 DoubleRow, catT8 pure comp stationary) ----
            # one PSUM tile per accumulation group: a start=True matmul
            # clears the whole bank, so groups must never share a live tile
            for tt in range(NT):
                for h in range(H):
                    psv = psA.tile([P, S], dt.float32, tag="psA")
                    for g in range(2):
                        nc.tensor.matmul(
                            psv[:, 0:DO],
                            catT8[:, ds(2 * g, 2), ts(tt, P)],
                            wv_sb[:, h, g], perf_mode=DR,
                            start=(g == 0), stop=(g == 1))
                    nc.vector.tensor_scalar_mul(v_sb[:, tt, h, :],
                                                psv[:, 0:DO], IWSC)
            # ---- transposed attention ----
            for h in range(H):
                for tt in range(NT):
                    pss = psA.tile([P, S], dt.float32, tag="psA")
                    nc.tensor.matmul(pss[:], kT[:, h, ts(tt, P)], qT[:, h, :],
                                     start=True, stop=True)
                    nc.scalar.activation(afT[:, tt, h, :], pss[:], AF.Exp,
                                         scale=SQS)
            # all-head row sums over t (partition axis) via indicator-matmul
            psS4 = psS.tile([4, S], dt.float32, tag="psS")
            nmm = 0
            for h in range(H):
                for tt in range(NT):
                    nc.tensor.matmul(psS4[:], hsel_sb[:, ds(h * H, H)],
                                     afT[:, tt, h, :],
                                     start=(nmm == 0), stop=(nmm == 15))
                    nmm += 1
            # reciprocal once on [4, S], cast fp16, broadcast via K=4 matmul
            rs4 = acts.tile([H, S], dt.float32)
            nc.vector.reciprocal(rs4[:], psS4[:])
            rs4h = acts.tile([H, S], dt.float16)
            nc.vector.tensor_copy(rs4h[:], rs4[:])
            for h in range(H):
                psB1 = psB.tile([P, S], dt.float32, tag="psB")
                nc.tensor.matmul(psB1[:], hbsel_sb[:, ds(h * P, P)],
                                 rs4h[:], start=True, stop=True)
                rsB = rsbp.tile([P, S], dt.float32, tag="rsB")
                nc.scalar.copy(rsB[:], psB1[:])
                psc = psA.tile([P, S], dt.float32, tag="psA")
                for tt in range(NT):
                    nc.tensor.matmul(psc[:], v_sb[:, tt, h, :],
                                     afT[:, tt, h, :],
                                     start=(tt == 0), stop=(tt == NT - 1))
                nc.vector.tensor_tensor(out=ctxT[:, h, :], in0=psc[:],
                                        in1=rsB[:], op=OP.mult)
            nc.leave_named_scope("heads", sc_hd[0], False)

            # ---- mha = LN(ctx_cat @ Wo) ----
            sc_mf = nc.enter_named_scope("mha_ffn", False)
            ps_mha = []
            for st in range(NT):
                psm = psA.tile([P, DI], dt.float32, tag="psA")
                for h in range(H):
                    nc.tensor.matmul(psm[:], ctxT[:, h, ts(st, P)],
                                     wo_sb[:, h, :],
                                     start=(h == 0), stop=(h == H - 1))
                ps_mha.append(psm)
            ln_rows([t[:] for t in ps_mha],
                    [mha_n[:, c, :] for c in range(NT)], eps1)
            for kt in range(KC):
                pst = psT.tile([P, S], dt.float16, tag="psT")
                for c in range(NT):
                    nc.tensor.transpose(pst[:, ts(c, P)],
                                        mha_n[:, c, ts(kt, P)], ident_sb[:])
                nc.vector.tensor_copy(mhaT16[:, kt, :], pst[:])

            # ---- FFN: W1 stationary so hdn lands transposed; ACT Gelu ----
            psh = psA.tile([P, S], dt.float32, tag="psA")
            for kt in range(KC):
                nc.tensor.matmul(psh[:], w1_sb[:, kt, :],
                                 mhaT16[:, kt, :],
                                 start=(kt == 0), stop=(kt == KC - 1))
            nc.scalar.activation(hdnT[:], psh[:], AF.Gelu)

            # ---- enc = LN(hdn @ W2) ----
            ps_enc = []
            for st in range(NT):
                pse = psA.tile([P, DI], dt.float32, tag="psA")
                nc.tensor.matmul(pse[:], hdnT[:, ts(st, P)], w2_sb[:],
                                 start=True, stop=True)
                ps_enc.append(pse)
            ln_rows([t[:] for t in ps_enc],
                    [enc_n[:, c, :] for c in range(NT)], eps1)
            for kt in range(KC):
                pst = psT.tile([P, S], dt.float16, tag="psT")
                for c in range(NT):
                    nc.tensor.transpose(pst[:, ts(c, P)],
                                        enc_n[:, c, ts(kt, P)], ident_sb[:])
                if kt % 2 == 0:
                    nc.vector.tensor_copy(encT8[:, kt, :], pst[:])
                else:
                    nc.scalar.copy(encT8[:, kt, :], pst[:])
            nc.leave_named_scope("mha_ffn", sc_mf[0], False)

        # ======================= vocab head =======================
        with ExitStack() as vctx:
            sc_vc = nc.enter_named_scope("vocab", False)
            slabp = vctx.enter_context(tc.tile_pool(name="slabp", bufs=1))
            stgp = vctx.enter_context(tc.tile_pool(name="stgp", bufs=4))
            scr2 = vctx.enter_context(tc.tile_pool(name="scr2", bufs=3))
            sstat = vctx.enter_context(tc.tile_pool(name="sstat", bufs=2))
            psV = vctx.enter_context(
                tc.tile_pool(name="psV", bufs=2, space="PSUM"))

            slab_a = slabp.tile([P, VPAD], dt.float16, tag="slab0")
            slab_b = slabp.tile([P, VPAD], dt.float16, tag="slab1")
            slabs = (slab_a, slab_b)

            def emit_piece(sh, lsums, j, act_sub=False, ring=None):
                """out = slab_logit - lsum, then DMA (gpsimd ring)."""
                off = j * PW
                plen = min(PW, V - off)
                for pc in range(2):
                    st = 2 * sh + pc
                    stg = stgp.tile([P, PW], dt.float16, tag="stg",
                                    name=f"stg{st}_{j}")
                    if act_sub and pc == 0:
                        nc.scalar.activation(
                            stg[:, :plen], slabs[pc][:, ds(off, plen)],
                            AF.Identity, bias=lsums[2])
                    else:
                        nc.vector.tensor_scalar_sub(
                            stg[:, :plen], slabs[pc][:, ds(off, plen)],
                            lsums[pc])
                    eng = ring if ring is not None else nc.gpsimd
                    eng.dma_start(
                        out_d[st * P:(st + 1) * P, off:off + plen],
                        stg[:, :plen])

            prev = None
            for sh in range(2):
                for pj in range(NPIECE):
                    if prev is not None:
                        emit_piece(0, prev, pj)
                    wts = []
                    for u in range(4):
                        wt = wtokp.tile([P, 2, 2, 512], dt.float8e4,
                                        tag="wtok")
                        nc.sync.dma_start(wt[:], wtok_d[pj * 4 + u])
                        wts.append(wt)
                    nv = PW if pj < NPIECE - 1 else V - (NPIECE - 1) * PW
                    for pc in range(2):
                        st = 2 * sh + pc
                        psl = psV.tile([P, PW], dt.float32, tag="psV")
                        for g in range(2):
                            for u in range(4):
                                nc.tensor.matmul(
                                    psl[:, ds(u * 512, 512)],
                                    encT8[:, ds(2 * g, 2), ts(st, P)],
                                    wts[u][:, g], perf_mode=DR,
                                    start=(g == 0), stop=(g == 1))
                        # single reader of psl (DVE copy) — the exp+accum
                        # reads the staged fp16 slab instead, so PSUM recycles
                        # at DVE pace and ACT runs off the critical path
                        nc.vector.tensor_scalar_mul(
                            slabs[pc][:, ds(pj * PW, nv)], psl[:, :nv],
                            IWSC)
                        scr = scr2.tile([P, PW], dt.float16, tag="escr")
                        nc.scalar.activation(
                            scr[:, :nv], slabs[pc][:, ds(pj * PW, nv)],
                            AF.Exp, accum_out=sums_sb[:, st, pj:pj + 1])
                # lsums for this half
                stot2 = sstat.tile([P, 2], dt.float32, tag="stot",
                                   name=f"stot{sh}")
                for pc in range(2):
                    st = 2 * sh + pc
                    nc.vector.reduce_sum(stot2[:, pc:pc + 1],
                                         sums_sb[:, st, :], axis=AX.X)
                lsum2 = sstat.tile([P, 2], dt.float32, tag="lsum",
                                   name=f"lsum{sh}")
                nc.scalar.activation(lsum2[:], stot2[:], AF.Ln)
                negl2 = sstat.tile([P, 1], dt.float32, tag="negl",
                                   name=f"negl{sh}")
                nc.vector.tensor_scalar_mul(negl2[:], lsum2[:, 0:1], -1.0)
                prev = [lsum2[:, 0:1], lsum2[:, 1:2], negl2[:, 0:1]]
            # tail: in-place subs on the slab (no staging buffer — the slab
            # is dead after this), DMA straight from slab on three rings
            rings = (nc.scalar, nc.sync)
            for j in range(NPIECE):
                off = j * PW
                plen = min(PW, V - off)
                for pc in range(2):
                    st = 2 + pc
                    nc.vector.tensor_scalar_sub(
                        slabs[pc][:, ds(off, plen)],
                        slabs[pc][:, ds(off, plen)], prev[pc])
                    eng = rings[(2 * j + pc) % 2]
                    eng.dma_start(
                        out_d[st * P:(st + 1) * P, off:off + plen],
                        slabs[pc][:, ds(off, plen)])
            nc.leave_named_scope("vocab", sc_vc[0], False)

    nc.compile()
    return nc


def get_program():
    if "nc" not in _CACHE:
        _CACHE["nc"] = build_program()
    return _CACHE["nc"]


def kernel(_trace=False, **inputs):
    from concourse.bass_utils import run_bass_kernel_spmd

    nc = get_program()
    shared = host_prep(inputs)
    in_maps = []
    for b in range(B):
        m = dict(shared)
        m.update(host_prep_core(inputs, b))
        in_maps.append(m)
    res = run_bass_kernel_spmd(nc, in_maps, list(range(B)), trace=_trace)
    _CACHE["last_res"] = res
    out = np.stack([np.asarray(res.results[b]["out"], np.float32)
                    for b in range(B)])
    if _trace:
        return out, res
    return out
